# revision 1
# baseline (speedup 1.0000x reference)
"""DiGCN-style 2-layer GCN message-passing kernel for 8 trn2 NeuronCores.

Contract: kernel(**inputs) takes FULL unsharded inputs (as produced by the
problem's setup_inputs) and returns the FULL [N, D] float32 output.

Strategy (per spec sharding hint):
 - Nodes sharded 12500/core (8 cores). Edges partitioned by destination node
   so the segment-sum aggregation is core-local.
 - Per layer, each core scales its activation shard by dinv (deg^-1/2),
   transposes it to node-major bf16, and the shards are AllGather'd (in 4
   quarter chunks) so every core holds the full gather table in HBM.
 - Messages are fetched with dma_gather (int16 indices -> 4 table banks of
   25088 rows each), and segment-summed on the TensorEngine by multiplying
   gathered edge blocks [128e x 128f] against host-built one-hot-with-weight
   matrices S_w [128e x width] accumulated into PSUM windows of 500 dst
   nodes.  dinv[col] is applied on PSUM->SBUF copy; the GCN weight transform
   and the parallel linear branch accumulate into a second PSUM window.
 - BatchNorm batch statistics are computed with bn_stats/bn_aggr locally and
   combined across cores with a tiny AllReduce; scale/shift(+ReLU) applied
   with one fused scalar-engine activation pass.
"""

import os
import sys

for _p in ("/opt/trn_rl_repo", os.path.expanduser("~/.axon_site/_ro/trn_rl_repo")):
    if os.path.isdir(_p) and _p not in sys.path:
        sys.path.insert(0, _p)

import numpy as np

import concourse.bass as bass
import concourse.bacc as bacc
import concourse.mybir as mybir
import concourse.tile as tile
from concourse.masks import make_identity

F32 = mybir.dt.float32
BF16 = mybir.dt.float16  # gather-path dtype (fp16: 11-bit mantissa)
I16 = mybir.dt.int16
NP_BF16 = mybir.dt.np(BF16)

P = 128  # partitions / feature dim


class Cfg:
    def __init__(self, n_nodes=100000, n_edges=625000, depth=2, bn_eps=1e-5,
                 n_cores=8, subwin=500, group_subwins=3):
        self.N = n_nodes
        self.E = n_edges
        self.depth = depth
        self.bn_eps = bn_eps
        self.C = n_cores
        self.SUBWIN = subwin                      # dst nodes per PSUM window
        self.NL = self.N // self.C                # nodes per core
        assert self.NL % subwin == 0
        self.NSUB = self.NL // subwin             # PSUM windows per core
        self.NT = (self.NL + P - 1) // P          # 128-node transpose tiles
        self.NLP = self.NT * P                    # padded nodes per core
        assert self.NLP % 4 == 0
        self.BANKROWS = 2 * self.NLP              # rows per gather bank (2 cores)
        self.NBANK = (self.C * self.NLP) // self.BANKROWS
        assert self.NBANK == 4
        assert self.BANKROWS <= 32768, "bank must fit int16 index range"
        self.GS = group_subwins                   # subwins per gather group
        self.groups = [list(range(g, min(g + self.GS, self.NSUB)))
                       for g in range(0, self.NSUB, self.GS)]


class Pre:
    """Host-side preprocessing output (program structure + per-core data)."""
    pass


def preprocess(x, edge_index, edge_weight, cfg: Cfg):
    c = cfg
    row = np.asarray(edge_index[0], dtype=np.int64)
    col = np.asarray(edge_index[1], dtype=np.int64)
    w = np.asarray(edge_weight, dtype=np.float32)

    core = col // c.NL
    dst_local = col % c.NL
    sub = dst_local // c.SUBWIN            # subwindow within core
    dcol = dst_local % c.SUBWIN            # column within subwindow

    src_core = row // c.NL
    src_local = row % c.NL
    table_row = src_core * c.NLP + src_local
    bank = table_row // c.BANKROWS
    idx_local = table_row % c.BANKROWS

    # sort edges by (core, sub, bank, dcol)
    order = np.lexsort((dcol, bank, sub, core))
    core, sub, bank, dcol, idx_local, w = (
        core[order], sub[order], bank[order], dcol[order], idx_local[order], w[order])

    # counts per (core, sub, bank)
    key = (core * c.NSUB + sub) * 4 + bank
    nbins = c.C * c.NSUB * 4
    counts = np.bincount(key, minlength=nbins).reshape(c.C, c.NSUB, 4)
    starts = np.zeros_like(counts)
    flat = counts.reshape(c.C, -1)
    st = np.cumsum(flat, axis=1) - flat
    core_base = np.concatenate([[0], np.cumsum(counts.sum(axis=(1, 2)))])[:-1]
    starts = (st + core_base[:, None]).reshape(c.C, c.NSUB, 4)

    # program-uniform block counts per (sub, bank)
    maxcnt = counts.max(axis=0)            # [NSUB, 4]
    nblk = np.maximum((maxcnt + P - 1) // P, 0)
    for s in range(c.NSUB):
        if nblk[s].sum() == 0:
            nblk[s][0] = 1                 # keep every window covered

    # column windows per (sub, bank, blk): union of per-core spans
    wins = {}
    for s in range(c.NSUB):
        first = True
        for b in range(4):
            for k in range(int(nblk[s][b])):
                if first:
                    wins[(s, b, k)] = (0, c.SUBWIN)
                    first = False
                    continue
                lo, hi = c.SUBWIN, 0
                for ci in range(c.C):
                    cnt = int(counts[ci, s, b])
                    r0, r1 = k * P, min((k + 1) * P, cnt)
                    if r1 <= r0:
                        continue
                    st0 = int(starts[ci, s, b])
                    dd = dcol[st0 + r0: st0 + r1]
                    lo = min(lo, int(dd.min()))
                    hi = max(hi, int(dd.max()) + 1)
                if hi <= lo:
                    lo, hi = 0, 1
                wins[(s, b, k)] = (lo, hi)

    # S_w stream layout: per sub (in order), per bank, per blk: [128, width]
    sw_off = {}
    off = 0
    sub_off = np.zeros(c.NSUB, dtype=np.int64)
    sub_w = np.zeros(c.NSUB, dtype=np.int64)
    for s in range(c.NSUB):
        sub_off[s] = off
        for b in range(4):
            for k in range(int(nblk[s][b])):
                sw_off[(s, b, k)] = off
                off += wins[(s, b, k)][1] - wins[(s, b, k)][0]
        sub_w[s] = off - sub_off[s]
    SW_TOT = int(off)

    # gather segments: per (group, bank) concat of padded (sub, bank) slot lists
    gb_T = {}      # (g,b) -> slot count (multiple of 128)
    gb_off = {}    # (g,b) -> offset (in slots/16 units) into idx tensor
    blk_pos = {}   # (s,b,k) -> (g, free-slot block index within (g,b))
    tot16 = 0
    for g, subs in enumerate(c.groups):
        for b in range(4):
            t = 0
            for s in subs:
                for k in range(int(nblk[s][b])):
                    blk_pos[(s, b, k)] = (g, t)
                    t += 1
            T = t * P
            gb_T[(g, b)] = T
            gb_off[(g, b)] = tot16
            tot16 += T // 16
    TOT16 = int(tot16)

    # per-core data arrays
    xs = []
    sws = []
    idxs = []
    for ci in range(c.C):
        # x shard, feature-major, padded
        xf = np.zeros((P, c.NLP), dtype=np.float32)
        xf[:, :c.NL] = np.asarray(x[ci * c.NL:(ci + 1) * c.NL]).T
        xs.append(xf)

        sw = np.zeros((P, SW_TOT), dtype=NP_BF16)
        idxa = np.zeros((P, TOT16), dtype=np.int16)
        for s in range(c.NSUB):
            for b in range(4):
                cnt = int(counts[ci, s, b])
                st0 = int(starts[ci, s, b])
                g, _ = blk_pos.get((s, b, 0), (None, None))
                for k in range(int(nblk[s][b])):
                    r0, r1 = k * P, min((k + 1) * P, cnt)
                    n = max(0, r1 - r0)
                    lo, hi = wins[(s, b, k)]
                    o = sw_off[(s, b, k)]
                    if n > 0:
                        rows = np.arange(n)
                        cc = dcol[st0 + r0: st0 + r0 + n] - lo
                        assert (cc >= 0).all() and (cc < hi - lo).all()
                        blkmat = np.zeros((P, hi - lo), dtype=np.float32)
                        blkmat[rows, cc] = w[st0 + r0: st0 + r0 + n]
                        sw[:, o:o + hi - lo] = blkmat.astype(NP_BF16)
                    # idx slots for this block
                    gg, tpos = blk_pos[(s, b, k)]
                    base_slot = gb_off[(gg, b)] * 16 + tpos * P
                    vals = np.zeros(P, dtype=np.int16)
                    if n > 0:
                        vals[:n] = idx_local[st0 + r0: st0 + r0 + n].astype(np.int16)
                    # slot j -> idx tensor [p, free]: free = base/16 + j//16, stream p = j%16
                    j = np.arange(P)
                    fr = (base_slot + j) // 16
                    pp = (base_slot + j) % 16
                    for rep in range(8):
                        idxa[rep * 16 + pp, fr] = vals
        sws.append(sw)
        idxs.append(idxa)

    # dense per-destination weight bins for the degree computation
    dcnt = np.zeros(c.N, dtype=np.int64)
    np.add.at(dcnt, col0 := np.asarray(edge_index[1], dtype=np.int64), 1)
    WROWS = int(min(max(dcnt.max(), 1), P))
    w_all = np.asarray(edge_weight, dtype=np.float32)
    wdl = []
    for ci in range(c.C):
        wd = np.zeros((WROWS, c.NSUB, c.SUBWIN), dtype=np.float32)
        wdl.append(wd)
    slot_of = np.zeros(c.E, dtype=np.int64)
    seen = np.zeros(c.N, dtype=np.int64)
    order2 = np.argsort(col0, kind="stable")
    cs = col0[order2]
    # slot index within each destination = running count
    slot_sorted = np.arange(c.E) - np.concatenate(
        [[0], np.cumsum(np.bincount(cs, minlength=c.N))])[cs]
    for ci in range(c.C):
        m = (cs // c.NL) == ci
        ee = order2[m]
        sl = np.minimum(slot_sorted[m], WROWS - 1)
        dl = cs[m] % c.NL
        np.add.at(wdl[ci], (sl, dl // c.SUBWIN, dl % c.SUBWIN), w_all[ee])
    pre = Pre()
    pre.WROWS = WROWS
    pre.w_dense = wdl
    pre.cfg = c
    pre.nblk = nblk
    pre.wins = wins
    pre.sw_off = sw_off
    pre.sub_off = sub_off
    pre.sub_w = sub_w
    pre.SW_TOT = SW_TOT
    pre.gb_T = gb_T
    pre.gb_off = gb_off
    pre.blk_pos = blk_pos
    pre.TOT16 = TOT16
    pre.x_shards = xs
    pre.sw_shards = sws
    pre.idx_shards = idxs
    return pre


def build_program(pre, debug=False, phase=99):
    # phase: int level or set of feature strings
    if isinstance(phase, int):
        feats = set()
        order = ["deg", "dinv", "table", "ag", "gather", "msg", "h"]
        for i, f in enumerate(order):
            if phase >= i + 2:
                feats.add(f)
        if phase >= 99:
            feats.add("layer2")
    else:
        feats = set(phase)
    c = pre.cfg
    nc = bacc.Bacc("TRN2", target_bir_lowering=False, debug=debug,
                   num_devices=c.C, num_swdge_queues=4)

    x_in = nc.dram_tensor("x_fm", [P, c.NLP], F32, kind="ExternalInput")
    sw_in = nc.dram_tensor("s_w", [P, max(pre.SW_TOT, 1)], BF16, kind="ExternalInput")
    idx_in = nc.dram_tensor("idx16", [P, max(pre.TOT16, 1)], I16, kind="ExternalInput")
    wd_in = nc.dram_tensor("w_dense", [pre.WROWS, c.NSUB, c.SUBWIN], F32,
                           kind="ExternalInput")
    wlin_in = nc.dram_tensor("w_lin_t", [P, c.depth, P], F32, kind="ExternalInput")
    wgcn_in = nc.dram_tensor("w_gcn_t", [P, c.depth, P], F32, kind="ExternalInput")
    gamma_in = nc.dram_tensor("gamma_t", [P, c.depth], F32, kind="ExternalInput")
    beta_in = nc.dram_tensor("beta_t", [P, c.depth], F32, kind="ExternalInput")
    out_t = nc.dram_tensor("out", [P, c.NL], F32, kind="ExternalOutput")

    rg = [list(range(c.C))]

    with tile.TileContext(nc) as tc:
        with (
            tc.tile_pool(name="const", bufs=1) as cp,
            tc.tile_pool(name="swp", bufs=2) as swp,
            tc.tile_pool(name="gat", bufs=2) as gat,
            tc.tile_pool(name="work", bufs=4) as wk,
            tc.tile_pool(name="small", bufs=4) as sm,
            tc.tile_pool(name="psum", bufs=2, space="PSUM") as pp,
            tc.tile_pool(name="psum1", bufs=2, space="PSUM") as pp1,
            tc.tile_pool(name="dram", bufs=1, space="DRAM") as dp,
        ):
            # ---------- persistent tiles ----------
            x = cp.tile([P, c.NLP], F32)
            nc.sync.dma_start(out=x[:], in_=x_in[:])
            idx_sb = cp.tile([P, max(pre.TOT16, 1)], I16)
            nc.sync.dma_start(out=idx_sb[:], in_=idx_in[:])
            ident = cp.tile([P, P], F32)
            make_identity(nc, ident[:])
            ones_colw = cp.tile([pre.WROWS, 1], F32)
            nc.vector.memset(ones_colw[:], 1.0)
            ones_row = cp.tile([1, P], F32)
            nc.vector.memset(ones_row[:], 1.0)
            wlin = cp.tile([P, c.depth, P], F32)
            nc.sync.dma_start(out=wlin[:], in_=wlin_in[:])
            wgcn = cp.tile([P, c.depth, P], F32)
            nc.sync.dma_start(out=wgcn[:], in_=wgcn_in[:])
            gamma = cp.tile([P, c.depth], F32)
            nc.sync.dma_start(out=gamma[:], in_=gamma_in[:])
            beta = cp.tile([P, c.depth], F32)
            nc.sync.dma_start(out=beta[:], in_=beta_in[:])
            dinv_bcast = cp.tile([P, c.NL], F32)
            dinv_ncol = cp.tile([P, c.NT], F32)
            if "dinv" not in feats:
                nc.vector.memset(dinv_bcast[:], 1.0)
                nc.vector.memset(dinv_ncol[:], 1.0)

            # dram scratch
            shard = dp.tile([c.NLP, P], BF16)
            tables_by_layer = [
                dp.tile([c.C * c.NLP, P], BF16, name=f"table{li}",
                        addr_space="Shared")
                for li in range(c.depth)]
            ddeg = dp.tile([1, c.NLP], F32)
            ddinv = dp.tile([1, c.NLP], F32)

            # ---------- degree ----------
            if 'deg' in feats:
             for s in range(c.NSUB):
                 wdt = sm.tile([pre.WROWS, c.SUBWIN], F32, tag="wdt", bufs=2)
                 nc.sync.dma_start(out=wdt[:], in_=wd_in[:, s, :])
                 dps = pp.tile([1, c.SUBWIN], F32, tag="msgp")
                 nc.tensor.matmul(out=dps[:], lhsT=ones_colw[:],
                                  rhs=wdt[:], start=True, stop=True)
                 drow = sm.tile([1, c.SUBWIN], F32, tag="drow", bufs=2)
                 nc.vector.tensor_copy(out=drow[:], in_=dps[:])
                 nc.sync.dma_start(
                     out=ddeg[:, s * c.SUBWIN:(s + 1) * c.SUBWIN], in_=drow[:])
             if c.NLP > c.NL:
                 zp = sm.tile([1, c.NLP - c.NL], F32, tag="zpad", bufs=2)
                 nc.vector.memset(zp[:], 0.0)
                 nc.sync.dma_start(out=ddeg[:, c.NL:], in_=zp[:])
            # read node-column layout [128, NT]
             deg_nc = sm.tile([P, c.NT], F32, tag="degnc")
             nc.sync.dma_start(out=deg_nc[:],
                               in_=ddeg[:].rearrange("one (t p) -> (one p) t", p=P))
             # dinv = (deg > 0) * 1/sqrt(max(deg, tiny))
             mask = sm.tile([P, c.NT], F32, tag="mask")
             nc.vector.tensor_scalar(out=mask[:], in0=deg_nc[:], scalar1=0.0,
                                     scalar2=None, op0=mybir.AluOpType.is_gt)
             nc.vector.tensor_scalar_max(out=deg_nc[:], in0=deg_nc[:], scalar1=1e-30)
             nc.scalar.activation(out=deg_nc[:], in_=deg_nc[:],
                                  func=mybir.ActivationFunctionType.Sqrt)
             nc.vector.reciprocal(out=deg_nc[:], in_=deg_nc[:])
             nc.vector.tensor_mul(out=dinv_ncol[:], in0=deg_nc[:], in1=mask[:])
             # roundtrip for free-major row + broadcast
             nc.sync.dma_start(out=ddinv[:].rearrange("one (t p) -> (one p) t", p=P),
                               in_=dinv_ncol[:])
             for s in range(c.NSUB):
                 dinv_row = sm.tile([1, c.SUBWIN], F32, tag="dinvrow", bufs=2)
                 nc.sync.dma_start(
                     out=dinv_row[:],
                     in_=ddinv[:, s * c.SUBWIN:(s + 1) * c.SUBWIN])
                 bps = pp.tile([P, c.SUBWIN], F32, tag="msgp")
                 nc.tensor.matmul(out=bps[:], lhsT=ones_row[:],
                                  rhs=dinv_row[:],
                                  start=True, stop=True)
                 nc.vector.tensor_copy(
                     out=dinv_bcast[:, s * c.SUBWIN:(s + 1) * c.SUBWIN], in_=bps[:])

            # ---------- layers ----------
            nlayers = c.depth if 'layer2' in feats else (1 if 'table' in feats else 0)
            for li in range(nlayers):
                tables = tables_by_layer[li]
                banks = [tables[b * c.BANKROWS:(b + 1) * c.BANKROWS, :]
                         for b in range(4)]
                # xs table: transpose x tiles, scale by dinv (per node), cast bf16
                for t in range(c.NT):
                    tp = pp1.tile([P, P], F32, tag="hp")
                    nc.tensor.transpose(out=tp[:], in_=x[:, t * P:(t + 1) * P],
                                        identity=ident[:])
                    stg = wk.tile([P, P], BF16, tag="stage")
                    nc.vector.tensor_scalar_mul(out=stg[:], in0=tp[:],
                                                scalar1=dinv_ncol[:, t:t + 1])
                    nc.sync.dma_start(out=shard[t * P:(t + 1) * P, :], in_=stg[:])
                # all-gather full shard -> full table
                if 'ag' in feats:
                    nc.gpsimd.collective_compute(
                        "AllGather", mybir.AluOpType.bypass, replica_groups=rg,
                        ins=[shard[:].opt()], outs=[tables[:].opt()])

                stats = sm.tile([P, c.NSUB], F32, tag="stats", bufs=1)
                stats2 = sm.tile([P, c.NSUB], F32, tag="stats2", bufs=1)

                for g, subs in (enumerate(c.groups) if 'gather' in feats else []):
                    gbufs = {}
                    for b in range(4):
                        T = pre.gb_T[(g, b)]
                        if T == 0:
                            continue
                        gb = gat.tile([P, T // P, P], BF16, tag=f"g{b}")
                        off16 = pre.gb_off[(g, b)]
                        nc.gpsimd.dma_gather(
                            gb[:], banks[b], idx_sb[:, off16:off16 + T // 16],
                            num_idxs=T, num_idxs_reg=T, elem_size=P, elem_step=P,
                            single_packet=False, queue_num=b)
                        gbufs[b] = gb
                    for s in (subs if 'msg' in feats else []):
                        swt = swp.tile([P, int(max(pre.sub_w.max(), 1))], BF16,
                                       tag="swt")
                        ww = int(pre.sub_w[s])
                        nc.sync.dma_start(
                            out=swt[:, :ww],
                            in_=sw_in[:, int(pre.sub_off[s]):int(pre.sub_off[s]) + ww])
                        msg = pp.tile([P, c.SUBWIN], F32, tag="msgp")
                        items = [(b, k) for b in range(4)
                                 for k in range(int(pre.nblk[s][b]))]
                        for j, (b, k) in enumerate(items):
                            lo, hi = pre.wins[(s, b, k)]
                            o = pre.sw_off[(s, b, k)] - int(pre.sub_off[s])
                            _, tpos = pre.blk_pos[(s, b, k)]
                            nc.tensor.matmul(
                                out=msg[:, lo:hi],
                                lhsT=gbufs[b][:, tpos, :],
                                rhs=swt[:, o:o + hi - lo],
                                start=(j == 0), stop=(j == len(items) - 1))
                        # agg = msg * dinv[col]
                        agg = wk.tile([P, c.SUBWIN], F32, tag="agg", bufs=2)
                        sl = slice(s * c.SUBWIN, (s + 1) * c.SUBWIN)
                        nc.vector.tensor_mul(out=agg[:], in0=msg[:],
                                             in1=dinv_bcast[:, sl])
                        if 'h' not in feats:
                            continue
                        # h = x @ W_lin.T + agg @ W_gcn.T
                        hp = pp1.tile([P, c.SUBWIN], F32, tag="hp")
                        nc.tensor.matmul(out=hp[:], lhsT=wlin[:, li, :], rhs=x[:, sl],
                                         start=True, stop=False)
                        nc.tensor.matmul(out=hp[:], lhsT=wgcn[:, li, :], rhs=agg[:],
                                         start=False, stop=True)
                        nc.vector.tensor_copy(out=x[:, sl], in_=hp[:])
                        nc.vector.tensor_reduce(
                            out=stats[:, s:s + 1], in_=x[:, sl],
                            axis=mybir.AxisListType.X, op=mybir.AluOpType.add)
                        sqd = wk.tile([P, c.SUBWIN], F32, tag="sqd", bufs=2)
                        nc.scalar.activation(
                            out=sqd[:], in_=x[:, sl],
                            func=mybir.ActivationFunctionType.Square,
                            accum_out=stats2[:, s:s + 1])

                if 'h' not in feats:
                    continue
                # batch-norm statistics across all nodes/cores
                ar = sm.tile([P, 2], F32, tag="ar")
                nc.vector.tensor_reduce(out=ar[:, 0:1], in_=stats[:],
                                        axis=mybir.AxisListType.X,
                                        op=mybir.AluOpType.add)
                nc.vector.tensor_reduce(out=ar[:, 1:2], in_=stats2[:],
                                        axis=mybir.AxisListType.X,
                                        op=mybir.AluOpType.add)
                st_in = dp.tile([P, 2], F32, tag=f"sti{li}")
                st_out = dp.tile([P, 2], F32, tag=f"sto{li}")
                nc.sync.dma_start(out=st_in[:], in_=ar[:])
                nc.gpsimd.collective_compute(
                    "AllReduce", mybir.AluOpType.add, replica_groups=rg,
                    ins=[st_in[:].opt()], outs=[st_out[:].opt()])
                gs = sm.tile([P, 2], F32, tag="gs")
                nc.sync.dma_start(out=gs[:], in_=st_out[:])
                mu = sm.tile([P, 1], F32, tag="mu")
                nc.vector.tensor_scalar_mul(out=mu[:], in0=gs[:, 0:1],
                                            scalar1=1.0 / c.N)
                esq = sm.tile([P, 1], F32, tag="esq")
                nc.vector.tensor_scalar_mul(out=esq[:], in0=gs[:, 1:2],
                                            scalar1=1.0 / c.N)
                mu2 = sm.tile([P, 1], F32, tag="mu2")
                nc.vector.tensor_mul(out=mu2[:], in0=mu[:], in1=mu[:])
                var = sm.tile([P, 1], F32, tag="var")
                nc.vector.tensor_sub(out=var[:], in0=esq[:], in1=mu2[:])
                nc.vector.tensor_scalar_add(out=var[:], in0=var[:],
                                            scalar1=c.bn_eps)
                nc.scalar.activation(out=var[:], in_=var[:],
                                     func=mybir.ActivationFunctionType.Sqrt)
                nc.vector.reciprocal(out=var[:], in_=var[:])
                A = sm.tile([P, 1], F32, tag="A")
                nc.vector.tensor_mul(out=A[:], in0=var[:], in1=gamma[:, li:li + 1])
                muA = sm.tile([P, 1], F32, tag="muA")
                nc.vector.tensor_mul(out=muA[:], in0=mu[:], in1=A[:])
                B = sm.tile([P, 1], F32, tag="B")
                nc.vector.tensor_sub(out=B[:], in0=beta[:, li:li + 1], in1=muA[:])
                fn = (mybir.ActivationFunctionType.Relu if li != c.depth - 1
                      else mybir.ActivationFunctionType.Identity)
                for s in range(c.NSUB):
                    sl = slice(s * c.SUBWIN, (s + 1) * c.SUBWIN)
                    nc.scalar.activation(out=x[:, sl], in_=x[:, sl], func=fn,
                                         bias=B[:], scale=A[:])

            nc.sync.dma_start(out=out_t[:], in_=x[:, :c.NL])

    nc.compile()
    return nc


def make_in_maps(inputs, pre):
    c = pre.cfg
    wlt = np.ascontiguousarray(
        np.transpose(np.asarray(inputs["W_lin"], np.float32), (2, 0, 1)))
    wgt = np.ascontiguousarray(
        np.transpose(np.asarray(inputs["W_gcn"], np.float32), (2, 0, 1)))
    gt = np.ascontiguousarray(np.asarray(inputs["gamma"], np.float32).T)
    bt = np.ascontiguousarray(np.asarray(inputs["beta"], np.float32).T)
    maps = []
    for ci in range(c.C):
        maps.append({
            "x_fm": pre.x_shards[ci],
            "s_w": pre.sw_shards[ci],
            "idx16": pre.idx_shards[ci],
            "w_dense": pre.w_dense[ci],
            "w_lin_t": wlt,
            "w_gcn_t": wgt,
            "gamma_t": gt,
            "beta_t": bt,
        })
    return maps


def assemble_output(results, cfg):
    outs = [np.asarray(r["out"]) for r in results]
    return np.concatenate([o.T for o in outs], axis=0).astype(np.float32)


def run(inputs, cfg=None, trace=False):
    from concourse import bass_utils
    cfg = cfg or Cfg()
    pre = preprocess(inputs["x"], inputs["edge_index"], inputs["edge_weight"], cfg)
    nc = build_program(pre)
    maps = make_in_maps(inputs, pre)
    res = bass_utils.run_bass_kernel_spmd(nc, maps, core_ids=list(range(cfg.C)),
                                          trace=trace)
    return assemble_output(res.results, cfg), res


def kernel(**inputs) -> np.ndarray:
    out, _ = run(inputs)
    return out

